# revision 1
# baseline (speedup 1.0000x reference)
"""Trainium2 Bass kernel for nn_Action_15942918602807 (pointer-gen / copy-net
action head with context scatter).

Sharding: data-parallel over batch B=32 across 8 NeuronCores (4 batches/core).

Per-core device pipeline (fp16 on the PE, fp32 softmax/assembly):
  gen_logit  = dec_out @ W_gen + b_gen            (PE, fp16, K=512 + K=1 bias row)
  copy_logit = dec_out @ src_hidden.T + mask_pen  (PE, fp32, K=512 + K=1 penalty)
  gen_exp    = exp(gen_logit * H^-1/2)            (ACT, accum_out -> row sums)
  copy_exp   = exp(copy_logit * H^-1/2)           (ACT, accum_out)
  Z = sum(gen_exp)+sum(copy_exp); R = 1/Z         (DVE)
  copy_w = copy_exp * R, transposed to lhsT       (DVE scale + PE transpose)
  out = gen_exp*R + copy_w.T @ [sources; onehot(ctx)]
        - sources = [pv_m; l_onehot; tp; related] as 2 K-tiles of 128 (fp16)
        - the context scatter-add is 2 more K-tiles whose rhs is a one-hot
          matrix built on the fly by DVE int16 iota==transfer compares
  final: out_chunk = gen_exp*R + copy_psum via one scalar_tensor_tensor (DVE)

Rows are packed in 32-partition strips (16 used + 16 zero) so the four
batches' copy matmuls land on distinct PE column groups (tile_position) and
one [128, N] DVE op assembles all four batches at once.
"""

import numpy as np

# problem constants (hardcoded per harness contract)
V = 30000
VP = 30720          # V padded to 30 groups x 1024
NG = 30             # column groups of 1024
GW = 1024           # group width
H = 512
B, L = 32, 16
SRC = 506           # 50 pv + 50 l + 50 tp + 256 ctx + 100 rel
C_LEN = 256
NB = 4              # batches per core
NCORES = 8
SCALE = float(H) ** -0.5

_CACHE = {}


def _build_program(debug=False, npasses=1):
    import concourse.bacc as bacc
    import concourse.mybir as mybir
    import concourse.tile as tile
    from concourse.masks import make_identity

    dt = mybir.dt
    Alu = mybir.AluOpType
    Act = mybir.ActivationFunctionType

    nc = bacc.Bacc(None, target_bir_lowering=False)

    # ---- I/O ----
    decT16_d = nc.dram_tensor("decT16", [4, 128, 128], dt.float16, kind="ExternalInput")
    decT32_d = nc.dram_tensor("decT32", [4, 128, 64], dt.float32, kind="ExternalInput")
    wgen_d = nc.dram_tensor("wgen16", [513, VP], dt.float16, kind="ExternalInput")
    s16_d = nc.dram_tensor("s16", [NB, 2, 128, VP], dt.float16, kind="ExternalInput")
    srcT_d = nc.dram_tensor("srcT32", [NB, 4, 128, SRC], dt.float32, kind="ExternalInput")
    tr_d = nc.dram_tensor("transfer32", [NB, 2, 128, 1], dt.int32, kind="ExternalInput")
    pen_d = nc.dram_tensor("maskpen32", [NB, 1, SRC], dt.float32, kind="ExternalInput")
    out_d = nc.dram_tensor("out", [NB, L, VP], dt.float32, kind="ExternalOutput")

    with tile.TileContext(nc) as tc:
        with (
            tc.tile_pool(name="const", bufs=1) as cpool,
            tc.tile_pool(name="w", bufs=4) as wpool,
            tc.tile_pool(name="bias", bufs=2) as bpool,
            tc.tile_pool(name="s", bufs=16) as spool,
            tc.tile_pool(name="oh", bufs=16) as ohpool,
            tc.tile_pool(name="srct", bufs=4) as srctpool,
            tc.tile_pool(name="outp", bufs=2) as outpool,
            tc.tile_pool(name="psg", bufs=2, space="PSUM") as psg,
            tc.tile_pool(name="psc", bufs=2, space="PSUM") as psc,
            tc.tile_pool(name="pscl", bufs=1, space="PSUM") as pscl,
            tc.tile_pool(name="pstr", bufs=1, space="PSUM") as pstr,
        ):
            # ---- constants / setup ----
            iota1024 = cpool.tile([128, GW], dt.float16)
            nc.gpsimd.iota(iota1024[:], pattern=[[1, GW]], base=0, channel_multiplier=0,
                           allow_small_or_imprecise_dtypes=True)
            iotag = cpool.tile([128, NG], dt.int32)
            nc.gpsimd.iota(iotag[:], pattern=[[GW, NG]], base=0, channel_multiplier=0)
            identity = cpool.tile([128, 128], dt.float32)
            make_identity(nc, identity[:])
            ones16 = cpool.tile([1, 128], dt.float16)
            nc.gpsimd.memset(ones16[:], 1.0)
            ones32 = cpool.tile([1, 16], dt.float32)
            nc.gpsimd.memset(ones32[:], 1.0)

            dt16 = cpool.tile([128, 4 * 128], dt.float16)   # gen lhsT, kt-major
            dt32 = cpool.tile([128, 4 * 64], dt.float32)    # copy_logit lhsT
            for kt in range(4):
                nc.sync.dma_start(out=dt16[:, 128 * kt:128 * (kt + 1)], in_=decT16_d[kt])
                nc.sync.dma_start(out=dt32[:, 64 * kt:64 * (kt + 1)], in_=decT32_d[kt])

            trs = cpool.tile([128, 8], dt.int32)            # transfer cols (b, ct)
            for b in range(NB):
                for ct in range(2):
                    nc.gpsimd.dma_start(out=trs[:, 2 * b + ct:2 * b + ct + 1], in_=tr_d[b, ct])
            shifted_i = cpool.tile([128, 8 * NG], dt.int32)  # transfer - 1024*g
            for b in range(NB):
                for ct in range(2):
                    i = 2 * b + ct
                    nc.vector.tensor_tensor(
                        out=shifted_i[:, NG * i:NG * (i + 1)],
                        in0=trs[:, i:i + 1].to_broadcast([128, NG]),
                        in1=iotag[:],
                        op=Alu.subtract,
                    )
            shifted = cpool.tile([128, 8 * NG], dt.float32)
            nc.vector.tensor_copy(out=shifted[:], in_=shifted_i[:])

            pen_sb = cpool.tile([1, NB * SRC], dt.float32)
            for b in range(NB):
                nc.gpsimd.dma_start(out=pen_sb[:, SRC * b:SRC * (b + 1)], in_=pen_d[b])

            # copy_exp columns: [0:250) dense+rel, [256:512) ctx — each
            # 128-col K-block starts 512B-aligned for the PE transpose
            copy_exp = cpool.tile([128, 512], dt.float32)
            nc.gpsimd.memset(copy_exp[:], 0.0)
            lhsT_all = cpool.tile([128, 512], dt.float16)   # 16 blocks of 32 (kt, b)
            nc.gpsimd.memset(lhsT_all[:], 0.0)

            gen_exp = cpool.tile([128, VP], dt.float16)
            gen_part = cpool.tile([128, 64], dt.float32)    # 60 gen + 2 copy partials
            nc.gpsimd.memset(gen_part[:], 0.0)
            z_t = cpool.tile([128, 1], dt.float32)
            r_t = cpool.tile([128, 1], dt.float32)

            for _pass in range(npasses):  # npasses>1 only for HW timing
                # ---- stage 1: copy_logit + exp (unscaled) ----
                for b in range(NB):
                    cl = pscl.tile([16, SRC], dt.float32, tag="cl")
                    for kt in range(4):
                        st = srctpool.tile([128, SRC], dt.float32, tag="st32")
                        nc.sync.dma_start(out=st[:], in_=srcT_d[b, kt])
                        nc.tensor.matmul(
                            out=cl[:], lhsT=dt32[:, 64 * kt + 16 * b:64 * kt + 16 * (b + 1)],
                            rhs=st[:], start=(kt == 0), stop=False,
                        )
                    nc.tensor.matmul(
                        out=cl[:], lhsT=ones32[:, 0:16], rhs=pen_sb[:, SRC * b:SRC * (b + 1)],
                        start=False, stop=True,
                    )
                    nc.scalar.activation(
                        out=copy_exp[32 * b:32 * b + 16, 0:250], in_=cl[:, 0:250],
                        func=Act.Exp, scale=SCALE,
                        accum_out=gen_part[32 * b:32 * b + 16, 60:61],
                    )
                    nc.scalar.activation(
                        out=copy_exp[32 * b:32 * b + 16, 256:512], in_=cl[:, 250:506],
                        func=Act.Exp, scale=SCALE,
                        accum_out=gen_part[32 * b:32 * b + 16, 61:62],
                    )

                # ---- stage 2: transpose unscaled copy probs into lhsT blocks ----
                # copy_exp columns are host-permuted: [pv;l;tp;rel;ctx], each
                # K-tile one contiguous 512B-aligned block
                jobs = [(0, 128, 0), (128, 256, 1), (256, 384, 2), (384, 512, 3)]
                for b in range(NB):
                    for (c0, c1, kt) in jobs:
                        pt = pstr.tile([128, 16], dt.float32, tag="pt")
                        nc.tensor.transpose(
                            out=pt[:], in_=copy_exp[32 * b:32 * b + 16, c0:c1],
                            identity=identity[32 * b:32 * b + 16, 32 * b:32 * b + 16],
                            tile_position=(32 * b, 0),
                        )
                        blk = 32 * (4 * kt + b)
                        nc.vector.tensor_copy(out=lhsT_all[:, blk:blk + 16], in_=pt[:])

                # ---- stage 3: interleaved gen + copy over column groups ----
                # gen_exp accumulates exp(gen_logit*s), then copy psum is merged
                # in-place (t = gen_exp + copy_psum); normalization deferred
                for gp in range(NG // 2):
                    # W for two groups
                    wts = []
                    for kt in range(4):
                        wt = wpool.tile([128, 2 * GW], dt.float16, tag="wt")
                        nc.sync.dma_start(
                            out=wt[:], in_=wgen_d[128 * kt:128 * (kt + 1), 2 * GW * gp:2 * GW * (gp + 1)])
                        wts.append(wt)
                    bt = bpool.tile([1, 2 * GW], dt.float16, tag="bt")
                    nc.sync.dma_start(out=bt[:], in_=wgen_d[512:513, 2 * GW * gp:2 * GW * (gp + 1)])
                    for gg in range(2):
                        g = 2 * gp + gg
                        # gen: two 512-chunks
                        for cix in range(2):
                            cs = GW * gg + 512 * cix
                            chunk = 2 * g + cix
                            pg = psg.tile([128, 512], dt.float32, tag="pg")
                            for kt in range(4):
                                nc.tensor.matmul(
                                    out=pg[:], lhsT=dt16[:, 128 * kt:128 * (kt + 1)],
                                    rhs=wts[kt][:, cs:cs + 512], start=(kt == 0), stop=False,
                                )
                            nc.tensor.matmul(
                                out=pg[:], lhsT=ones16[:], rhs=bt[:, cs:cs + 512],
                                start=False, stop=True,
                            )
                            nc.scalar.activation(
                                out=gen_exp[:, 512 * chunk:512 * (chunk + 1)], in_=pg[:],
                                func=Act.Exp, scale=SCALE,
                                accum_out=gen_part[:, chunk:chunk + 1],
                            )
                        # copy: S tiles + onehot for this group
                        sts = []
                        for b in range(NB):
                            for kt in range(2):
                                st = spool.tile([128, GW], dt.float16, tag="st")
                                nc.sync.dma_start(out=st[:], in_=s16_d[b, kt, :, GW * g:GW * (g + 1)])
                                sts.append(st)
                        ohs = []
                        for b in range(NB):
                            for ct in range(2):
                                oh = ohpool.tile([128, GW], dt.float16, tag="oh")
                                i = 2 * b + ct
                                eng = nc.vector
                                eng.tensor_scalar(
                                    out=oh[:], in0=iota1024[:],
                                    scalar1=shifted[:, NG * i + g:NG * i + g + 1], scalar2=None,
                                    op0=Alu.is_equal,
                                )
                                ohs.append(oh)
                        for cix in range(2):
                            cs = 512 * cix
                            chunk = 2 * g + cix
                            # bank A: strips b0 (colgrp 0) / b1 (colgrp 32)
                            # bank B: strips b2 (colgrp 64) / b3 (colgrp 96)
                            pcA = psc.tile([128, 512], dt.float32, tag="pcA")
                            pcB = psc.tile([128, 512], dt.float32, tag="pcB")
                            for pair in range(2):          # (b0,b2) then (b1,b3)
                                for kt in range(4):
                                    for half in range(2):  # interleave banks
                                        b = 2 * half + pair
                                        pc = pcA if half == 0 else pcB
                                        if kt < 2:
                                            rhs = sts[2 * b + kt][:, cs:cs + 512]
                                        else:
                                            rhs = ohs[2 * b + (kt - 2)][:, cs:cs + 512]
                                        blk = 32 * (4 * kt + b)
                                        nc.tensor.matmul(
                                            out=pc[32 * b:32 * (b + 1), :],
                                            lhsT=lhsT_all[:, blk:blk + 32],
                                            rhs=rhs, start=(kt == 0), stop=(kt == 3),
                                            tile_position=(0, 32 * b),
                                        )
                            nc.vector.tensor_tensor(
                                out=gen_exp[0:64, 512 * chunk:512 * (chunk + 1)],
                                in0=gen_exp[0:64, 512 * chunk:512 * (chunk + 1)],
                                in1=pcA[0:64, :], op=Alu.add,
                            )
                            nc.vector.tensor_tensor(
                                out=gen_exp[64:128, 512 * chunk:512 * (chunk + 1)],
                                in0=gen_exp[64:128, 512 * chunk:512 * (chunk + 1)],
                                in1=pcB[64:128, :], op=Alu.add,
                            )

                # ---- stage 4: Z, R ----
                nc.vector.reduce_sum(out=z_t[:], in_=gen_part[:, 0:62], axis=mybir.AxisListType.X)
                nc.vector.reciprocal(out=r_t[:], in_=z_t[:])

                # ---- stage 5: normalize + store ----
                for gp in range(NG // 2):
                    ot = outpool.tile([128, 2 * GW], dt.float32, tag="ot")
                    for cix in range(4):
                        cs = 512 * cix
                        chunk = 4 * gp + cix
                        nc.vector.tensor_scalar(
                            out=ot[:, cs:cs + 512],
                            in0=gen_exp[:, 512 * chunk:512 * (chunk + 1)],
                            scalar1=r_t[:, 0:1], scalar2=None, op0=Alu.mult,
                        )
                    for b in range(NB):
                        nc.sync.dma_start(out=out_d[b, :, 2 * GW * gp:2 * GW * (gp + 1)],
                                          in_=ot[32 * b:32 * b + 16, :])

            if debug:
                dbg_ce = nc.dram_tensor("dbg_copy_exp", [128, 512], dt.float32, kind="ExternalOutput")
                dbg_lh = nc.dram_tensor("dbg_lhsT", [128, 512], dt.float16, kind="ExternalOutput")
                dbg_r = nc.dram_tensor("dbg_r", [128, 2], dt.float32, kind="ExternalOutput")
                dbg_gp = nc.dram_tensor("dbg_genpart", [128, 64], dt.float32, kind="ExternalOutput")
                dbg_ge = nc.dram_tensor("dbg_genexp", [128, VP], dt.float16, kind="ExternalOutput")
                dbg_io = nc.dram_tensor("dbg_iota", [128, GW], dt.float16, kind="ExternalOutput")
                dbg_sh = nc.dram_tensor("dbg_shift", [128, 8 * NG], dt.float32, kind="ExternalOutput")
                nc.sync.dma_start(out=dbg_io[:], in_=iota1024[:])
                nc.sync.dma_start(out=dbg_sh[:], in_=shifted[:])
                nc.sync.dma_start(out=dbg_ce[:], in_=copy_exp[:])
                nc.sync.dma_start(out=dbg_lh[:], in_=lhsT_all[:])
                nc.sync.dma_start(out=dbg_r[:, 0:1], in_=r_t[:])
                nc.sync.dma_start(out=dbg_r[:, 1:2], in_=z_t[:])
                nc.sync.dma_start(out=dbg_gp[:], in_=gen_part[:])
                nc.sync.dma_start(out=dbg_ge[:], in_=gen_exp[:])

    nc.compile()
    return nc


PERM = np.concatenate([np.arange(150), np.arange(406, 506), np.arange(150, 406)])


def _prep_core_inputs(c, dec_out, src_hidden, src_mask, pv_m, l_onehot, tp,
                      related_topics, transfer):
    i0 = NB * c
    decT16 = np.zeros((512, 128), np.float16)
    decT32 = np.zeros((512, 64), np.float32)
    srcT32 = np.zeros((NB, 4, 128, SRC), np.float32)
    s16 = np.zeros((NB, 2, 128, VP), np.float16)
    tr32 = np.zeros((NB, 2, 128, 1), np.int32)
    pen32 = np.zeros((NB, 1, SRC), np.float32)
    for b in range(NB):
        i = i0 + b
        dT = dec_out[i].T  # [512, 16]
        decT16[:, 32 * b:32 * b + 16] = dT.astype(np.float16)
        decT32[:, 16 * b:16 * (b + 1)] = dT
        sT = src_hidden[i].T[:, PERM]  # [512, 506], copy-source order permuted
        for kt in range(4):
            srcT32[b, kt] = sT[128 * kt:128 * (kt + 1)]
        s16[b, 0, 0:50, :V] = pv_m[i].astype(np.float16)
        s16[b, 0, 50:100, :V] = l_onehot[i].astype(np.float16)
        s16[b, 0, 100:128, :V] = tp[i, 0:28].astype(np.float16)
        s16[b, 1, 0:22, :V] = tp[i, 28:50].astype(np.float16)
        s16[b, 1, 22:122, :V] = related_topics[i].astype(np.float16)
        tr32[b] = transfer[i].astype(np.int32).reshape(2, 128, 1)
        pen32[b, 0] = np.where(src_mask[i, 0, PERM] == 0, np.float32(-1e9), np.float32(0))
    # reshape decT to kt-major [4, 128, *]
    return {
        "decT16": np.ascontiguousarray(decT16.reshape(4, 128, 128)),
        "decT32": np.ascontiguousarray(decT32.reshape(4, 128, 64)),
        "s16": s16,
        "srcT32": srcT32,
        "transfer32": tr32,
        "maskpen32": pen32,
    }


def kernel(dec_out, src_hidden, src_mask, pv_m, l_onehot, tp, related_topics,
           context, glo2loc, W_gen, b_gen):
    from concourse.bass_utils import run_bass_kernel_spmd

    dec_out = np.asarray(dec_out, np.float32)
    src_hidden = np.asarray(src_hidden, np.float32)
    src_mask = np.asarray(src_mask, np.float32)
    W_gen = np.asarray(W_gen, np.float32)
    b_gen = np.asarray(b_gen, np.float32)
    in_dtype = np.asarray(context).dtype

    if "nc" not in _CACHE:
        _CACHE["nc"] = _build_program()
    nc = _CACHE["nc"]

    wgen16 = np.zeros((513, VP), np.float16)
    wgen16[0:512, :V] = W_gen.astype(np.float16)
    wgen16[512, :V] = b_gen.astype(np.float16)
    wgen16[512, V:] = np.float16(-60000.0)  # pad cols: exp -> 0

    transfer = np.asarray(glo2loc)[np.asarray(context)]  # [B, C_LEN]

    in_maps = []
    for c in range(NCORES):
        m = _prep_core_inputs(c, dec_out, src_hidden, src_mask,
                              np.asarray(pv_m), np.asarray(l_onehot),
                              np.asarray(tp), np.asarray(related_topics), transfer)
        m["wgen16"] = wgen16
        in_maps.append(m)

    res = run_bass_kernel_spmd(nc, in_maps, list(range(NCORES)))

    out = np.empty((B, L, V), np.float32)
    for c in range(NCORES):
        out[NB * c:NB * (c + 1)] = res.results[c]["out"][:, :, :V]
    return out

